# revision 42
# baseline (speedup 1.0000x reference)
"""Causal multi-head attention (B=4, S=1024, E=1024, H=16) on 8 trn2 cores.

Sharding: core c -> (batch b = c//2, head-group hg = c%2 of 8 heads).
Each core computes its heads' QKV from x[b], causal softmax attention
(att weights written normalized), and a partial output projection over its
512 e-columns.  Host sums the two partials per batch and adds out_b.
"""

import threading

import numpy as np

B, S, E, H = 4, 1024, 1024, 16
HD = E // H          # 64
NCORES = 8
NH = 8               # local heads per core
P = 128
NT = S // P          # 8 sequence tiles
SCALE = HD ** -0.5   # 0.125
NEG = -1.0e30

_lock = threading.Lock()
_cache = {}


def _build_program():
    import concourse.mybir as mybir
    from concourse import bacc
    from concourse.tile import TileContext
    from concourse.masks import make_identity

    f32 = mybir.dt.float32
    f32r = mybir.dt.float32r
    Exp = mybir.ActivationFunctionType.Exp

    nc = bacc.Bacc()

    xb_d = nc.declare_dram_parameter("xb", [S, E], f32, isOutput=False)
    wqk_d = nc.declare_dram_parameter("wqkT", [E, 2 * NH * HD], f32r, isOutput=False)
    wv_d = nc.declare_dram_parameter("wvT", [E, NH * HD], f32r, isOutput=False)
    ow_d = nc.declare_dram_parameter("owT", [NH * HD, E], f32r, isOutput=False)
    bqk_d = nc.declare_dram_parameter("bqk", [1, 2 * NH * HD], f32r, isOutput=False)
    bv_d = nc.declare_dram_parameter("bv", [1, NH * HD], f32r, isOutput=False)
    att_d = nc.declare_dram_parameter("att", [NH, S, S], f32, isOutput=True)
    outp_d = nc.declare_dram_parameter("outp", [S, E], f32, isOutput=True)

    with TileContext(nc) as tc, (
        tc.tile_pool(name="const", bufs=1)) as cp, (
        tc.tile_pool(name="persist", bufs=1)) as pp, (
        tc.tile_pool(name="xload", bufs=2)) as xlp, (
        tc.tile_pool(name="ps", bufs=1, space="PSUM")) as psp, (
        tc.tile_pool(name="attn", bufs=2)) as anp, (
        tc.tile_pool(name="attd", bufs=6)) as adp, (
        tc.tile_pool(name="small", bufs=4)) as smp:

        # ---- constants ----
        ident = cp.tile([P, P], f32)
        make_identity(nc, ident)
        cmask = cp.tile([P, P], f32)    # [s, t]: NEG where t > s
        nc.gpsimd.memset(cmask, 0.0)
        nc.gpsimd.affine_select(
            out=cmask, in_=cmask, compare_op=mybir.AluOpType.is_ge,
            fill=NEG, base=0, pattern=[[-1, P]], channel_multiplier=1)
        cmaskT = cp.tile([P, P], f32)   # [t, s]: NEG where t > s
        nc.gpsimd.memset(cmaskT, 0.0)
        nc.gpsimd.affine_select(
            out=cmaskT, in_=cmaskT, compare_op=mybir.AluOpType.is_ge,
            fill=NEG, base=0, pattern=[[1, P]], channel_multiplier=-1)
        ones_f = cp.tile([1, 512], f32)
        nc.vector.memset(ones_f, 1.0)
        ones_row = cp.tile([1, 512], f32r)
        nc.vector.tensor_copy(ones_row[:], ones_f[:])
        ones_v = cp.tile([P, NH * NT], f32)
        nc.vector.memset(ones_v, 1.0)
        bf16 = mybir.dt.bfloat16
        ident_b = cp.tile([P, P], bf16)
        nc.vector.tensor_copy(ident_b[:], ident[:])
        cmask_b = cp.tile([P, P], bf16)
        nc.vector.tensor_copy(cmask_b[:], cmask[:])

        # ---- persistent tensors ----
        qkT = pp.tile([P, 8, S], f32r)
        vt = pp.tile([P, NT, NH, HD + 1], f32r)
        attnT = pp.tile([P, 4, S], f32r)
        bqk_s = pp.tile([1, 2 * NH * HD], f32r)
        bv_s = pp.tile([1, NH * HD], f32r)
        nc.vector.tensor_copy(
            vt[:, :, :, HD:HD + 1],
            ones_v[:].rearrange("p (a b c) -> p a b c", a=NT, b=NH, c=1))

        # ---- prep pool: xT + qkv weights, freed mid-program ----
        prep_ctx = tc.tile_pool(name="prep", bufs=1)
        prp = prep_ctx.__enter__()
        wqkT_s = prp.tile([P, 8, 2 * NH * HD], f32r)
        wvT_s = prp.tile([P, 8, NH * HD], f32r)
        xT = prp.tile([P, 8, S], f32r)

        # ---- x loads first; weight DMAs issued before evac copies so the
        # ACT HWDGE queue doesn't stall them behind transpose-gated copies
        xns = []
        for st in range(NT):
            xn = xlp.tile([P, E], f32, tag="xn", bufs=4)
            xns.append(xn)
            nc.scalar.dma_start(out=xn[:], in_=xb_d[st * P:(st + 1) * P, :])
        nc.scalar.dma_start(
            out=wqkT_s, in_=wqk_d[:].rearrange("(et p) f -> p et f", p=P))
        nc.scalar.dma_start(
            out=wvT_s, in_=wv_d[:].rearrange("(et p) f -> p et f", p=P))
        nc.sync.dma_start(out=bqk_s[:], in_=bqk_d[:])
        nc.sync.dma_start(out=bv_s[:], in_=bv_d[:])
        for st in range(NT):
            for et in range(8):
                tp = psp.tile([P, P], f32, tag="mm5", bufs=4)
                nc.tensor.transpose(tp, xns[st][:, et * P:(et + 1) * P], ident)
                nc.scalar.copy(xT[:, et, st * P:(st + 1) * P], tp)

        def qk_proj_chunk(ft, sc):
            ps = psp.tile([P, 512], f32, tag="mm5", bufs=4)
            if True:
                for et in range(8):
                    nc.tensor.matmul(
                        ps, wqkT_s[:, et, ft * P:(ft + 1) * P],
                        xT[:, et, sc * 512:(sc + 1) * 512],
                        start=(et == 0), stop=False)
                nc.tensor.matmul(
                    ps, bqk_s[:, ft * P:(ft + 1) * P], ones_row,
                    start=False, stop=True)
                nc.vector.tensor_copy(qkT[:, ft, sc * 512:(sc + 1) * 512], ps)

        def qk_proj(ft):
            qk_proj_chunk(ft, 0)
            qk_proj_chunk(ft, 1)

        def v_proj(tt):
            ps = psp.tile([P, 512], f32, tag="mm5", bufs=4)
            for et in range(8):
                nc.tensor.matmul(
                    ps, xT[:, et, tt * P:(tt + 1) * P],
                    wvT_s[:, et, :], start=(et == 0), stop=False)
            nc.tensor.matmul(
                ps, ones_row[:, 0:P], bv_s, start=False, stop=True)
            nc.vector.tensor_copy(
                vt[:, tt, :, 0:HD], ps[:].rearrange("p (h d) -> p h d", h=NH))

        def heads_of(h):
            bp = (h % 2) * HD
            return (bp, qkT[bp:bp + HD, h // 2, :], qkT[bp:bp + HD, 4 + h // 2, :])

        def nat_tiles(h, i0, i1, rs, ri):
            # natural scores -> exp(+rowsum) -> normalize -> att_weights out
            bp, qh, kh = heads_of(h)
            for i in range(i0, i1):
                live = (i + 1) * P
                pn = psp.tile([P, S], f32, tag="big", bufs=2)
                for c0 in range(0, live, 512):
                    w = min(512, live - c0)
                    nc.tensor.matmul(
                        pn[:, c0:c0 + w], qh[:, i * P:(i + 1) * P],
                        kh[:, c0:c0 + w], start=True, stop=True)
                nc.vector.tensor_add(pn[:, i * P:live], pn[:, i * P:live], cmask)
                en = anp.tile([P, S], f32, tag="en")
                nc.scalar.activation(
                    en[:, 0:live], pn[:, 0:live], Exp,
                    scale=SCALE, accum_out=rs[:, i:i + 1])
                nc.vector.reciprocal(ri[:, i:i + 1], rs[:, i:i + 1])
                ad = adp.tile([P, S], f32, tag="ad")
                nc.vector.tensor_scalar_mul(
                    ad[:, 0:live], en[:, 0:live], ri[:, i:i + 1])
                if live < S:
                    nc.gpsimd.memset(ad[:, live:S], 0.0)
                nc.sync.dma_start(
                    out=att_d[h, i * P:(i + 1) * P, :], in_=ad[:])

        def nat_pass(h):
            rs = smp.tile([P, NT], f32, tag="rs")
            ri = smp.tile([P, NT], f32, tag="ri")
            nat_tiles(h, 0, NT, rs, ri)

        lph = {}

        def pv_pass(h):
            # transposed scores -> exp -> PV (unnormalized) -> attnT
            bp, qh, kh = heads_of(h)
            po = psp.tile([HD + 1, S], f32, tag="big", bufs=2)
            for j in range(NT):
                liveS = S - j * P
                et_t = lph["lp"].tile([P, S], f32r, tag="et", bufs=6)
                for c0 in range(0, liveS, 512):
                    w = min(512, liveS - c0)
                    pt = psp.tile([P, 512], f32, tag="mm5", bufs=4)
                    nc.tensor.matmul(
                        pt[:, 0:w], kh[:, j * P:(j + 1) * P],
                        qh[:, j * P + c0:j * P + c0 + w],
                        start=True, stop=True)
                    if c0 == 0:
                        nc.vector.tensor_add(pt[:, 0:P], pt[:, 0:P], cmaskT)
                    nc.scalar.activation(
                        et_t[:, c0:c0 + w], pt[:, 0:w], Exp, scale=SCALE)
                for c in range(2):
                    lo = max(512 * c, j * P)
                    hi = 512 * (c + 1)
                    if lo >= hi:
                        continue
                    nc.tensor.matmul(
                        po[:, lo:hi], vt[:, j, h, :],
                        et_t[:, lo - j * P:hi - j * P],
                        start=(j == 0), stop=(j == (3 if c == 0 else 7)),
                        skip_group_check=True)
            rrow = lph["lp"].tile([1, S], f32, tag="rrow", bufs=2)
            nc.vector.tensor_copy(rrow, po[HD:HD + 1, :])
            rinvr = lph["lp"].tile([1, S], f32r, tag="rinvr", bufs=2)
            with nc.allow_low_precision(reason="f32r bits == f32"):
                nc.vector.reciprocal(rinvr, rrow)
            bc = lph["lp"].tile([P, S], f32, tag="bc", bufs=2)
            for c in range(2):
                pb = psp.tile([P, 512], f32, tag="mm5", bufs=4)
                nc.tensor.matmul(
                    pb[0:HD, :], ones_row[:, 0:HD],
                    rinvr[:, c * 512:(c + 1) * 512], start=True, stop=True)
                nc.vector.tensor_copy(
                    bc[bp:bp + HD, c * 512:(c + 1) * 512], pb[0:HD, :])
            nc.vector.tensor_mul(
                attnT[bp:bp + HD, h // 2, :], po[0:HD, :], bc[bp:bp + HD, :])

        # interleaved schedule: att DMA stream stays fed to the end
        qk_proj_chunk(0, 0); qk_proj_chunk(4, 0)
        rs0 = smp.tile([P, NT], f32, tag="rs")
        ri0 = smp.tile([P, NT], f32, tag="ri")
        nat_tiles(0, 0, 4, rs0, ri0)
        qk_proj_chunk(0, 1); qk_proj_chunk(4, 1)
        nat_tiles(0, 4, 8, rs0, ri0)
        for tt in range(0, 4):
            v_proj(tt)
        qk_proj(1); qk_proj(5)
        nat_pass(1)
        for tt in range(4, 8):
            v_proj(tt)
        qk_proj(2); qk_proj(6)
        qk_proj(3); qk_proj(7)
        nat_pass(2)
        prep_ctx.__exit__(None, None, None)

        late = tc.tile_pool(name="late", bufs=1)
        lp = late.__enter__()
        lph["lp"] = lp
        owT = lp.tile([P, 4, E], f32r)
        nc.scalar.dma_start(
            out=owT[:], in_=ow_d[:].rearrange("(et p) f -> p et f", p=P))
        pv_pass(0); nat_pass(3)
        pv_pass(1); nat_pass(4)
        pv_pass(2); nat_pass(5)
        rs6 = smp.tile([P, NT], f32, tag="rs")
        ri6 = smp.tile([P, NT], f32, tag="ri")
        rs7 = smp.tile([P, NT], f32, tag="rs")
        ri7 = smp.tile([P, NT], f32, tag="ri")
        pv_pass(3)
        nat_tiles(6, 0, 4, rs6, ri6)
        pv_pass(4)
        nat_tiles(6, 4, 8, rs6, ri6)
        pv_pass(5)
        nat_tiles(7, 0, 3, rs7, ri7)
        pv_pass(6)
        nat_tiles(7, 3, 6, rs7, ri7)
        pv_pass(7)
        nat_tiles(7, 6, 8, rs7, ri7)

        # ---- output projection (reuses pn psum slot) ----
        for i in range(NT):
            pso = psp.tile([P, E], f32, tag="big", bufs=2)
            for fc in range(2):
                for et in range(4):
                    nc.tensor.matmul(
                        pso[:, fc * 512:(fc + 1) * 512],
                        attnT[:, et, i * P:(i + 1) * P],
                        owT[:, et, fc * 512:(fc + 1) * 512],
                        start=(et == 0), stop=(et == 3))
            ot = adp.tile([P, E], f32, tag="ad")
            nc.scalar.copy(ot[:], pso[:])
            nc.scalar.dma_start(out=outp_d[i * P:(i + 1) * P, :], in_=ot[:])

        late.__exit__(None, None, None)

    nc.compile()
    return nc



def _get_runner():
    """Build program + a reusable jitted SPMD executor (compile once)."""
    with _lock:
        if "runner" in _cache:
            return _cache["runner"]

        import jax
        import numpy as np
        from jax.sharding import Mesh, PartitionSpec
        from jax.experimental.shard_map import shard_map
        import concourse.mybir as mybir
        from concourse import bass2jax

        nc = _build_program()
        bass2jax.install_neuronx_cc_hook()

        partition_name = (nc.partition_id_tensor.name
                          if nc.partition_id_tensor else None)
        in_names, out_names, out_avals, zero_outs = [], [], [], []
        for alloc in nc.m.functions[0].allocations:
            if not isinstance(alloc, mybir.MemoryLocationSet):
                continue
            name = alloc.memorylocations[0].name
            if alloc.kind == "ExternalInput":
                if name != partition_name:
                    in_names.append(name)
            elif alloc.kind == "ExternalOutput":
                shape = tuple(alloc.tensor_shape)
                dt = mybir.dt.np(alloc.dtype)
                out_names.append(name)
                out_avals.append(jax.core.ShapedArray(shape, dt))
                zero_outs.append(np.zeros(shape, dt))

        n_params = len(in_names)
        donate = tuple(range(n_params, n_params + len(zero_outs)))
        bind_in_names = list(in_names) + list(out_names)
        if partition_name is not None:
            bind_in_names.append(partition_name)

        def _body(*args):
            operands = list(args)
            if partition_name is not None:
                operands.append(bass2jax.partition_id_tensor())
            outs = bass2jax._bass_exec_p.bind(
                *operands,
                out_avals=tuple(out_avals),
                in_names=tuple(bind_in_names),
                out_names=tuple(out_names),
                lowering_input_output_aliases=(),
                sim_require_finite=True,
                sim_require_nnan=True,
                nc=nc,
            )
            return tuple(outs)

        devices = jax.devices()[:NCORES]
        mesh = Mesh(np.asarray(devices), ("core",))
        in_specs = (PartitionSpec("core"),) * (n_params + len(zero_outs))
        out_specs = (PartitionSpec("core"),) * len(out_names)
        sharded = jax.jit(
            shard_map(_body, mesh=mesh, in_specs=in_specs,
                      out_specs=out_specs, check_rep=False),
            keep_unused=True,
        )

        def execute(in_maps):
            per_core = [[m[nm] for nm in in_names] for m in in_maps]
            concat_in = [
                np.concatenate([per_core[c][i] for c in range(NCORES)], axis=0)
                for i in range(n_params)
            ]
            concat_zero = [
                np.zeros((NCORES * z.shape[0], *z.shape[1:]), z.dtype)
                for z in zero_outs
            ]
            out_arrs = sharded(*concat_in, *concat_zero)
            out_arrs = [np.asarray(a) for a in out_arrs]
            return [
                {nm: out_arrs[i].reshape(NCORES, *out_avals[i].shape)[c]
                 for i, nm in enumerate(out_names)}
                for c in range(NCORES)
            ]

        _cache["runner"] = (nc, execute)
        return _cache["runner"]


def make_in_maps(x, in_proj_weight, in_proj_bias, out_w):
    x = np.asarray(x, np.float32)
    w = np.asarray(in_proj_weight, np.float32)
    bias = np.asarray(in_proj_bias, np.float32)
    ow = np.asarray(out_w, np.float32)
    in_maps = []
    for c in range(NCORES):
        b, hg = divmod(c, 2)
        qs = slice(hg * 512, hg * 512 + 512)
        ks = slice(E + hg * 512, E + hg * 512 + 512)
        vs = slice(2 * E + hg * 512, 2 * E + hg * 512 + 512)
        wqkT = np.ascontiguousarray(
            np.concatenate([w[qs], w[ks]], axis=0).T)          # [1024, 1024]
        wvT = np.ascontiguousarray(w[vs].T)                     # [1024, 512]
        owT = np.ascontiguousarray(ow[:, hg * 512:hg * 512 + 512].T)  # [512,1024]
        bqk = np.concatenate([bias[qs], bias[ks]])[None, :]
        bv = bias[vs][None, :]
        in_maps.append({
            "xb": np.ascontiguousarray(x[b]),
            "wqkT": wqkT, "wvT": wvT, "owT": owT,
            "bqk": np.ascontiguousarray(bqk), "bv": np.ascontiguousarray(bv),
        })
    return in_maps


def assemble(results, out_b):
    out = np.empty((B, S, E), np.float32)
    att = np.empty((B * H, S, S), np.float32)
    for c in range(NCORES):
        b, hg = divmod(c, 2)
        att[b * H + hg * NH: b * H + hg * NH + NH] = results[c]["att"]
    ob = np.asarray(out_b, np.float32)
    for b in range(B):
        out[b] = results[2 * b]["outp"] + results[2 * b + 1]["outp"] + ob[None, :]
    return out, att


def kernel(x, mask, in_proj_weight, in_proj_bias, out_w, out_b):
    _, execute = _get_runner()
    in_maps = make_in_maps(x, in_proj_weight, in_proj_bias, out_w)
    results = execute(in_maps)
    return assemble(results, out_b)
